# revision 6
# baseline (speedup 1.0000x reference)
"""Trainium2 Bass kernel for nn_Linear_18494129177115 (moe_routing).

Math (reference, fp32):
  base   = x @ W^T                                  [B,T,O]
  logits = x @ Wr^T + lang_bias                     [B,T,E]
  gates  = scatter(softmax(top2(logits)))           [B,T,E]
  h      = x @ A_e^T  (all experts)                 [B,T,E,R]
  out    = base + SCALING * sum_e gates_e * h_e @ B_e^T

Key identity used here: with A_cat = concat_e(A_e) [E*R, D] and
B_cat[e*R+r, o] = B[e, o, r], the whole gated LoRA collapses to
  out = x @ W^T + (gates_expanded * (x @ A_cat^T)) @ (SCALING * B_cat)
i.e. two extra thin matmuls fused into the base GEMM's PSUM accumulation.

Sharding: data-parallel over tokens, 1024 tokens/core on 8 cores; all
weights replicated. Each core's tokens lie in a single batch row, so the
language bias is a per-core constant row vector (fed as a tiny input).
No collectives.

On-device layout (per core): tokens on PSUM partitions, D contracted in
32 chunks of 128. x^T is SBUF-resident; W^T streams from HBM once.
Matmuls run as float32r (full-rate fp32 path on TRN2 PE); router logits
use plain float32 matmuls so the top-2 selection stays faithful to the
fp32 reference.
"""

import numpy as np

LANG_BIAS = 5.0
SCALING = 32.0 / 16.0
B_SZ, T_SZ, D_SZ, O_SZ, E_SZ, R_SZ = 4, 2048, 4096, 4096, 8, 16
NCORES = 8
TPC = (B_SZ * T_SZ) // NCORES      # 1024 tokens per core
NT = TPC // 128                    # 8 token tiles per core
NK = D_SZ // 128                   # 32 contraction chunks
NO = O_SZ // 512                   # 8 output tiles of 512
ER = E_SZ * R_SZ                   # 128 (expert, rank) pairs

_CACHE: dict = {}
LAST_RESULT = None


def _build_bass(loop_n: int | None = None):
    import concourse.bacc as bacc
    import concourse.mybir as mybir
    from concourse import tile
    from concourse.masks import make_identity

    f32 = mybir.dt.float32
    f32r = mybir.dt.float32r
    AX = mybir.AxisListType.X
    OP = mybir.AluOpType
    ACT = mybir.ActivationFunctionType

    nc = bacc.Bacc(None, target_bir_lowering=False, debug=False)

    xt_d = nc.dram_tensor("xt", [NK, 128, TPC], f32r, kind="ExternalInput")
    wt_d = nc.dram_tensor("wt", [NK, NO, 128, 512], f32r, kind="ExternalInput")
    acat_d = nc.dram_tensor("acat", [NK, 128, ER], f32r, kind="ExternalInput")
    wrt_d = nc.dram_tensor("wrt", [NK, 128, E_SZ], f32, kind="ExternalInput")
    bcat_d = nc.dram_tensor("bcat", [ER, O_SZ], f32r, kind="ExternalInput")
    bias_d = nc.dram_tensor("biasx", [128, E_SZ], f32, kind="ExternalInput")
    sel_d = nc.dram_tensor("sel", [E_SZ, ER], f32, kind="ExternalInput")
    out_d = nc.dram_tensor("out", [NT, 128, NO, 512], f32, kind="ExternalOutput")

    with tile.TileContext(nc) as tc:
        with (
            tc.tile_pool(name="const", bufs=1) as cpool,
            tc.tile_pool(name="wstream", bufs=4) as wpool,
            tc.tile_pool(name="ostage", bufs=3) as opool,
            tc.tile_pool(name="gate", bufs=2) as gpool,
            tc.tile_pool(name="psum", bufs=8, space="PSUM") as psum,
        ):

            def body(_iv=None):
                # ---- resident inputs ----
                xt_sb = cpool.tile([128, NK, TPC], f32r, name="xt_sb")
                acat_sb = cpool.tile([128, NK, ER], f32r, name="acat_sb")
                wrt_sb = cpool.tile([128, NK, E_SZ], f32, name="wrt_sb")
                bcat_sb = cpool.tile([ER, O_SZ], f32r, name="bcat_sb")
                bias_sb = cpool.tile([128, E_SZ], f32, name="bias_sb")
                sel_sb = cpool.tile([E_SZ, ER], f32, name="sel_sb")
                ident_sb = cpool.tile([128, 128], f32, name="ident_sb")
                hT_sb = cpool.tile([128, TPC], f32, name="hT_sb")
                ghT_sb = cpool.tile([128, NT, 128], f32r, name="ghT_sb")

                for kc in range(NK):
                    nc.sync.dma_start(xt_sb[:, kc, :], xt_d[kc])
                nc.sync.dma_start(acat_sb[:], acat_d[:].rearrange("k p e -> p k e"))
                nc.sync.dma_start(wrt_sb[:], wrt_d[:].rearrange("k p e -> p k e"))
                nc.sync.dma_start(bcat_sb[:], bcat_d[:])
                nc.sync.dma_start(bias_sb[:], bias_d[:])
                nc.sync.dma_start(sel_sb[:], sel_d[:])
                make_identity(nc, ident_sb[:])

                # ---- phase 1a: h^T = A_cat @ x^T   [ER, TPC], float32r ----
                for tb in range(TPC // 512):
                    ph = psum.tile([128, 512], f32, tag="bank", name=f"ph{tb}")
                    for kc in range(NK):
                        nc.tensor.matmul(
                            ph[:],
                            acat_sb[:, kc, :],
                            xt_sb[:, kc, tb * 512 : (tb + 1) * 512],
                            start=(kc == 0),
                            stop=(kc == NK - 1),
                        )
                    nc.vector.tensor_copy(hT_sb[:, tb * 512 : (tb + 1) * 512], ph[:])

                # ---- phase 1b: router logits + top-2 softmax gates ----
                for tt in range(NT):
                    ts = slice(tt * 128, (tt + 1) * 128)
                    pl = psum.tile([128, E_SZ], f32, tag="bank", name=f"pl{tt}")
                    for kc in range(NK):
                        nc.tensor.matmul(
                            pl[:],
                            xt_sb[:, kc, ts].bitcast(f32),
                            wrt_sb[:, kc, :],
                            start=(kc == 0),
                            stop=(kc == NK - 1),
                        )
                    logit = gpool.tile([128, E_SZ], f32, name="logit")
                    nc.vector.tensor_tensor(logit[:], pl[:], bias_sb[:], op=OP.add)

                    m1 = gpool.tile([128, 1], f32, name="m1")
                    nc.vector.reduce_max(m1[:], logit[:], axis=AX)
                    mask1 = gpool.tile([128, E_SZ], f32, name="mask1")
                    nc.vector.tensor_scalar(
                        mask1[:], logit[:], m1[:], None, op0=OP.is_equal
                    )
                    l2 = gpool.tile([128, E_SZ], f32, name="l2")
                    nc.vector.tensor_scalar(l2[:], mask1[:], -1e30, None, op0=OP.mult)
                    nc.vector.tensor_tensor(l2[:], l2[:], logit[:], op=OP.add)
                    m2 = gpool.tile([128, 1], f32, name="m2")
                    nc.vector.reduce_max(m2[:], l2[:], axis=AX)
                    mask2 = gpool.tile([128, E_SZ], f32, name="mask2")
                    nc.vector.tensor_scalar(
                        mask2[:], l2[:], m2[:], None, op0=OP.is_equal
                    )
                    w1 = gpool.tile([128, 1], f32, name="w1")
                    nc.scalar.activation(
                        w1[:], m2[:], ACT.Sigmoid, bias=m1[:], scale=-1.0
                    )
                    w2 = gpool.tile([128, 1], f32, name="w2")
                    nc.vector.tensor_scalar(
                        w2[:], w1[:], -1.0, 1.0, op0=OP.mult, op1=OP.add
                    )
                    g1 = gpool.tile([128, E_SZ], f32, name="g1")
                    nc.vector.tensor_scalar(g1[:], mask1[:], w1[:], None, op0=OP.mult)
                    gates = gpool.tile([128, E_SZ], f32, name="gates")
                    nc.vector.tensor_scalar(
                        gates[:], mask2[:], w2[:], None, op0=OP.mult
                    )
                    nc.vector.tensor_tensor(gates[:], gates[:], g1[:], op=OP.add)

                    # gates [t,E] -> gates^T [E,t] -> expand to [ER,t] -> gh^T
                    ptr = psum.tile([E_SZ, 128], f32, tag="bank", name=f"ptr{tt}")
                    nc.tensor.transpose(ptr[:], gates[:], ident_sb[:])
                    gT = gpool.tile([E_SZ, 128], f32, name="gT")
                    nc.vector.tensor_copy(gT[:], ptr[:])
                    pge = psum.tile([128, 128], f32, tag="bank", name=f"pge{tt}")
                    nc.tensor.matmul(pge[:], sel_sb[:], gT[:], start=True, stop=True)
                    nc.vector.tensor_tensor(
                        ghT_sb[:, tt, :], pge[:], hT_sb[:, ts], op=OP.mult
                    )

                # ---- phase 2: out = x @ W^T (+ gh @ SCALING*B_cat) ----
                for ot in range(NO):
                    po = [
                        psum.tile([128, 512], f32, tag="bank", name=f"po{ot}_{i}")
                        for i in range(NT)
                    ]
                    for kc in range(NK):
                        w_t = wpool.tile([128, 512], f32r, name="w_t")
                        nc.sync.dma_start(w_t[:], wt_d[kc, ot])
                        for tt in range(NT):
                            nc.tensor.matmul(
                                po[tt][:],
                                xt_sb[:, kc, tt * 128 : (tt + 1) * 128],
                                w_t[:],
                                start=(kc == 0),
                                stop=False,
                            )
                    for tt in range(NT):
                        nc.tensor.matmul(
                            po[tt][:],
                            ghT_sb[:, tt, :],
                            bcat_sb[:, ot * 512 : (ot + 1) * 512],
                            start=False,
                            stop=True,
                        )
                        ob = opool.tile([128, 512], f32, name="ob")
                        nc.vector.tensor_copy(ob[:], po[tt][:])
                        nc.sync.dma_start(out_d[tt, :, ot, :], ob[:])

            if loop_n is None:
                body()
            else:
                with tc.For_i(0, loop_n, 1) as iv:
                    body(iv)

    nc.compile()
    return nc


def _host_prep(x, language_ids, W, Wr, A, B):
    x = np.asarray(x, dtype=np.float32)
    W = np.asarray(W, dtype=np.float32)
    Wr = np.asarray(Wr, dtype=np.float32)
    A = np.asarray(A, dtype=np.float32)
    B = np.asarray(B, dtype=np.float32)
    lang = np.asarray(language_ids).astype(np.int64)

    xf = np.ascontiguousarray(x.reshape(B_SZ * T_SZ, D_SZ))
    # W^T [D,O] split into [kc, o_tile, 128, 512] contiguous DMA blocks
    wt = np.ascontiguousarray(W.T.reshape(NK, 128, NO, 512).transpose(0, 2, 1, 3))
    acat = A.reshape(ER, D_SZ)                       # row e*R+r = A[e,r]
    acat_t = np.ascontiguousarray(acat.T).reshape(NK, 128, ER)
    wrt = np.ascontiguousarray(Wr.T).reshape(NK, 128, E_SZ)
    bcat = np.ascontiguousarray(
        (SCALING * B.transpose(0, 2, 1)).reshape(ER, O_SZ)
    )                                                # [e*R+r, o] = SCALING*B[e,o,r]
    sel = np.zeros((E_SZ, ER), dtype=np.float32)
    sel[np.arange(ER) // R_SZ, np.arange(ER)] = 1.0

    in_maps = []
    for c in range(NCORES):
        shard = xf[c * TPC : (c + 1) * TPC]
        xt = np.ascontiguousarray(shard.T).reshape(NK, 128, TPC)
        b = int(lang[(c * TPC) // T_SZ])
        brow = np.zeros(E_SZ, dtype=np.float32)
        if b >= 0:
            brow[b] = LANG_BIAS
        bias = np.ascontiguousarray(np.tile(brow, (128, 1)))
        in_maps.append(
            {
                "xt": xt,
                "wt": wt,
                "acat": acat_t,
                "wrt": wrt,
                "bcat": bcat,
                "biasx": bias,
                "sel": sel,
            }
        )
    return in_maps


def kernel(x, language_ids, W, Wr, A, B):
    global LAST_RESULT
    from concourse.bass_utils import run_bass_kernel_spmd

    if "nc" not in _CACHE:
        _CACHE["nc"] = _build_bass()
    nc = _CACHE["nc"]

    in_maps = _host_prep(x, language_ids, W, Wr, A, B)
    res = run_bass_kernel_spmd(nc, in_maps, core_ids=list(range(NCORES)))
    LAST_RESULT = res
    outs = [r["out"].reshape(TPC, O_SZ) for r in res.results]
    return np.concatenate(outs, axis=0).reshape(B_SZ, T_SZ, O_SZ)
